# revision 5
# baseline (speedup 1.0000x reference)
import numpy as np
import jax
import jax.numpy as jnp

# Problem constants (hardcoded per spec)
B, L, D, N = 4, 4096, 1024, 512
LN_EPS = 1e-5
CH = 128          # chunk length
NC = L // CH      # 32 chunks
NDEV = 8
DS = D // NDEV    # channels per device

_IDX = np.arange(CH)[:, None] - np.arange(CH)[None, :]
_MASK = (_IDX >= 0)[:, :, None]
_IDXC = np.clip(_IDX, 0, CH - 1)

_BF16 = jnp.bfloat16
_F32 = jnp.float32


def _dss_dev(q, p, m, Tr, Ti, EPr, EPi, Ctr, Cti, Kl, g, bta, Dv):
    # q: [B,L,DS] int8   p,m: [B,L] f16 (rowscale*rstd, mu*rstd)
    # Tr/Ti: [CH+1,N] bf16 (exp(lam*k))   EPr/i: [N] f32   Ctr/i: [N,DS] f16
    # Kl: [CH,DS] f16    g,bta,Dv: [DS] f32
    u = q.astype(_F32) * p.astype(_F32)[..., None] - m.astype(_F32)[..., None]
    u = u * g + bta                               # [B,L,DS] f32
    ub = u.astype(_BF16)
    uc = ub.reshape(B, NC, CH, DS)

    A1r = jnp.flip(Tr[:CH], 0)                    # [s,n]: exp(lam*(CH-1-s))
    A1i = jnp.flip(Ti[:CH], 0)
    E2r = Tr[1:CH + 1]                            # [t,n]: exp(lam*(t+1))
    E2i = Ti[1:CH + 1]

    # local chunk states: Sloc[c,n,b,d] = sum_s A1[s,n] u[b,c,s,d]
    Slr = jnp.einsum('sn,bcsd->cnbd', A1r, uc, preferred_element_type=_F32)
    Sli = jnp.einsum('sn,bcsd->cnbd', A1i, uc, preferred_element_type=_F32)

    # scan over chunks: emitted state at step c covers chunks < c
    def step(carry, sl):
        sr, si = carry
        slr, sli = sl
        nsr = EPr[:, None, None] * sr - EPi[:, None, None] * si + slr
        nsi = EPr[:, None, None] * si + EPi[:, None, None] * sr + sli
        return (nsr, nsi), (sr, si)
    z = jnp.zeros((N, B, DS), _F32)
    _, (Spr, Spi) = jax.lax.scan(step, (z, z), (Slr, Sli))   # [NC,N,B,DS]

    Cr = Ctr.astype(_F32)[None, :, None, :]
    Ci = Cti.astype(_F32)[None, :, None, :]
    Wr = (Cr * Spr - Ci * Spi).astype(_BF16)
    Wi = (Cr * Spi + Ci * Spr).astype(_BF16)

    y_int = (jnp.einsum('tn,cnbd->bctd', E2r, Wr, preferred_element_type=_F32)
             - jnp.einsum('tn,cnbd->bctd', E2i, Wi, preferred_element_type=_F32))

    Ttoe = jnp.where(_MASK, Kl.astype(_BF16)[_IDXC, :], 0)   # [t,s,d]
    y_intra = jnp.einsum('tsd,bcsd->bctd', Ttoe, uc, preferred_element_type=_F32)

    y = (y_int + y_intra).reshape(B, L, DS) + u * Dv
    yrm = jnp.maximum(jnp.max(jnp.abs(y), axis=-1), 1e-30)   # [B,L]
    qy = jnp.rint(y * (127.0 / yrm)[..., None]).astype(jnp.int8)
    return qy, (yrm * (1.0 / 127.0)).astype(jnp.float16)


_CACHE = {}


def _get_fn():
    if 'fn' not in _CACHE:
        _CACHE['fn'] = jax.jit(_dss_dev)
    return _CACHE['fn']


def kernel(x, Lambda_real, Lambda_imag, C_real, C_imag, param_D, ln_gamma, ln_beta):
    f16 = np.float16
    import ml_dtypes
    bf16 = ml_dtypes.bfloat16

    x32 = np.asarray(x, np.float32)

    # ---- LayerNorm stats + int8 row quantization on host ----
    s1 = x32.sum(-1)
    s2 = np.einsum('bld,bld->bl', x32, x32, optimize=True)
    mu = s1 / D
    var = s2 / D - mu * mu
    rstd = 1.0 / np.sqrt(var + LN_EPS)            # [B,L]
    rm = np.maximum(x32.max(-1), -x32.min(-1))    # [B,L] abs-max per row
    rm = np.maximum(rm, 1e-30)
    inv_scale = 127.0 / rm
    q = np.rint(x32 * inv_scale[..., None]).astype(np.int8)   # [B,L,D]
    p16 = ((rm / 127.0) * rstd).astype(f16)
    m16 = (mu * rstd).astype(f16)

    # ---- kernel tables (float64 host precompute) ----
    lam = -np.exp(np.asarray(Lambda_real, np.float64)) \
        + 1j * np.exp(np.asarray(Lambda_imag, np.float64))     # [N]
    Cc = np.asarray(C_real, np.float64) + 1j * np.asarray(C_imag, np.float64)
    Ct = (Cc * (np.exp(lam) - 1.0) / lam).T                    # [N,D]

    k = np.arange(CH + 1)
    T = np.exp(lam[None, :] * k[:, None])                      # [CH+1,N]
    Tr = np.ascontiguousarray(np.real(T), bf16)
    Ti = np.ascontiguousarray(np.imag(T), bf16)
    EP = T[CH]
    EPr = np.real(EP).astype(np.float32)
    EPi = np.imag(EP).astype(np.float32)
    Kloc = np.real(T[:CH] @ Ct)                                # [CH,D]

    Dv = np.asarray(param_D, np.float32)
    g = np.asarray(ln_gamma, np.float32)
    bta = np.asarray(ln_beta, np.float32)

    fn = _get_fn()
    devs = jax.devices()[:NDEV]

    # ---- stage per-device args, ship in chain order, dispatch async ----
    puts, shards = [], []
    for i, dev in enumerate(devs):
        sl = slice(i * DS, (i + 1) * DS)
        args = (np.ascontiguousarray(q[:, :, sl]), p16, m16, Tr, Ti, EPr, EPi,
                np.ascontiguousarray(np.real(Ct[:, sl]), f16),
                np.ascontiguousarray(np.imag(Ct[:, sl]), f16),
                np.ascontiguousarray(Kloc[:, sl], f16),
                np.ascontiguousarray(g[sl]), np.ascontiguousarray(bta[sl]),
                np.ascontiguousarray(Dv[sl]))
        puts.extend(args)
        shards.extend([dev] * len(args))
    nargs = 13
    dbufs = jax.device_put(puts, shards)
    outs = [fn(*dbufs[i * nargs:(i + 1) * nargs]) for i in range(NDEV)]
    ys = jax.device_get(outs)                     # per dev: (int8 [B,L,DS], f16 [B,L])

    out = np.empty((B, L, D), np.float32)
    for i, (qy, rs) in enumerate(ys):
        out[:, :, i * DS:(i + 1) * DS] = qy * rs.astype(np.float32)[:, :, None]
    return out


# revision 6
# speedup vs baseline: 2.1704x; 2.1704x over previous
import threading
import numpy as np
import jax
import jax.numpy as jnp

# Problem constants (hardcoded per spec)
B, L, D, N = 4, 4096, 1024, 512
LN_EPS = 1e-5
CH = 128          # chunk length
NC = L // CH      # 32 chunks
NDEV = 8
DS = D // NDEV    # channels per device

_IDX = np.arange(CH)[:, None] - np.arange(CH)[None, :]
_MASK = (_IDX >= 0)[:, :, None]
_IDXC = np.clip(_IDX, 0, CH - 1)

_BF16 = jnp.bfloat16
_F32 = jnp.float32


def _dss_dev(q, p, m, Tr, Ti, EPr, EPi, Ctr, Cti, Kl, g, bta, Dv):
    # q: [B,L,DS] int8   p,m: [B,L] f16 (rowscale*rstd, mu*rstd)
    # Tr/Ti: [CH+1,N] bf16 (exp(lam*k))   EPr/i: [N] f32   Ctr/i: [N,DS] f16
    # Kl: [CH,DS] f16    g,bta,Dv: [DS] f32
    u = q.astype(_F32) * p.astype(_F32)[..., None] - m.astype(_F32)[..., None]
    u = u * g + bta                               # [B,L,DS] f32
    ub = u.astype(_BF16)
    uc = ub.reshape(B, NC, CH, DS)

    A1r = jnp.flip(Tr[:CH], 0)                    # [s,n]: exp(lam*(CH-1-s))
    A1i = jnp.flip(Ti[:CH], 0)
    E2r = Tr[1:CH + 1]                            # [t,n]: exp(lam*(t+1))
    E2i = Ti[1:CH + 1]

    # local chunk states: Sloc[c,n,b,d] = sum_s A1[s,n] u[b,c,s,d]
    Slr = jnp.einsum('sn,bcsd->cnbd', A1r, uc, preferred_element_type=_F32)
    Sli = jnp.einsum('sn,bcsd->cnbd', A1i, uc, preferred_element_type=_F32)

    # scan over chunks: emitted state at step c covers chunks < c
    def step(carry, sl):
        sr, si = carry
        slr, sli = sl
        nsr = EPr[:, None, None] * sr - EPi[:, None, None] * si + slr
        nsi = EPr[:, None, None] * si + EPi[:, None, None] * sr + sli
        return (nsr, nsi), (sr, si)
    z = jnp.zeros((N, B, DS), _F32)
    _, (Spr, Spi) = jax.lax.scan(step, (z, z), (Slr, Sli))   # [NC,N,B,DS]

    Cr = Ctr.astype(_F32)[None, :, None, :]
    Ci = Cti.astype(_F32)[None, :, None, :]
    Wr = (Cr * Spr - Ci * Spi).astype(_BF16)
    Wi = (Cr * Spi + Ci * Spr).astype(_BF16)

    y_int = (jnp.einsum('tn,cnbd->bctd', E2r, Wr, preferred_element_type=_F32)
             - jnp.einsum('tn,cnbd->bctd', E2i, Wi, preferred_element_type=_F32))

    Ttoe = jnp.where(_MASK, Kl.astype(_BF16)[_IDXC, :], 0)   # [t,s,d]
    y_intra = jnp.einsum('tsd,bcsd->bctd', Ttoe, uc, preferred_element_type=_F32)

    y = (y_int + y_intra).reshape(B, L, DS) + u * Dv
    yrm = jnp.maximum(jnp.max(jnp.abs(y), axis=-1), 1e-30)   # [B,L]
    qy = jnp.rint(y * (127.0 / yrm)[..., None]).astype(jnp.int8)
    return qy, (yrm * (1.0 / 127.0)).astype(jnp.float16)


_CACHE = {}


def _get_fn():
    if 'fn' not in _CACHE:
        _CACHE['fn'] = jax.jit(_dss_dev)
    return _CACHE['fn']


def kernel(x, Lambda_real, Lambda_imag, C_real, C_imag, param_D, ln_gamma, ln_beta):
    f16 = np.float16
    import ml_dtypes
    bf16 = ml_dtypes.bfloat16

    x32 = np.asarray(x, np.float32)

    # ---- LayerNorm stats + int8 row quantization on host ----
    s1 = x32.sum(-1)
    s2 = np.einsum('bld,bld->bl', x32, x32, optimize=True)
    mu = s1 / D
    var = s2 / D - mu * mu
    rstd = 1.0 / np.sqrt(var + LN_EPS)            # [B,L]
    rm = np.maximum(np.maximum(x32.max(-1), -x32.min(-1)), 1e-30)
    inv_scale = (127.0 / rm)[:, :, None]
    p16 = ((rm / 127.0) * rstd).astype(f16)
    m16 = (mu * rstd).astype(f16)

    # ---- kernel tables (float64 host precompute) ----
    lam = -np.exp(np.asarray(Lambda_real, np.float64)) \
        + 1j * np.exp(np.asarray(Lambda_imag, np.float64))     # [N]
    Cc = np.asarray(C_real, np.float64) + 1j * np.asarray(C_imag, np.float64)
    Ct = (Cc * (np.exp(lam) - 1.0) / lam).T                    # [N,D]

    k = np.arange(CH + 1)
    T = np.exp(lam[None, :] * k[:, None])                      # [CH+1,N]
    Tr = np.ascontiguousarray(np.real(T), bf16)
    Ti = np.ascontiguousarray(np.imag(T), bf16)
    EPr = np.real(T[CH]).astype(np.float32)
    EPi = np.imag(T[CH]).astype(np.float32)
    Kloc = np.real(T[:CH] @ Ct)                                # [CH,D]
    Ctr = np.real(Ct).astype(f16)
    Cti = np.imag(Ct).astype(f16)
    Kl16 = Kloc.astype(f16)

    Dv = np.asarray(param_D, np.float32)
    g = np.asarray(ln_gamma, np.float32)
    bta = np.asarray(ln_beta, np.float32)

    fn = _get_fn()
    devs = jax.devices()[:NDEV]
    out = np.empty((B, L, D), np.float32)

    def fetch(i, fut):
        qy, rs = jax.device_get(fut)
        out[:, :, i * DS:(i + 1) * DS] = qy * rs.astype(np.float32)[:, :, None]

    threads = []
    for i, dev in enumerate(devs):
        sl = slice(i * DS, (i + 1) * DS)
        q_i = np.rint(x32[:, :, sl] * inv_scale).astype(np.int8)
        args = [q_i, p16, m16, Tr, Ti, EPr, EPi,
                np.ascontiguousarray(Ctr[:, sl]),
                np.ascontiguousarray(Cti[:, sl]),
                np.ascontiguousarray(Kl16[:, sl]),
                np.ascontiguousarray(g[sl]), np.ascontiguousarray(bta[sl]),
                np.ascontiguousarray(Dv[sl])]
        d = jax.device_put(args, [dev] * len(args))
        fut = fn(*d)
        th = threading.Thread(target=fetch, args=(i, fut))
        th.start()
        threads.append(th)
    for th in threads:
        th.join()
    return out
